# revision 7
# baseline (speedup 1.0000x reference)
"""Trainium2 Bass kernel for nn_ALNet (adaptive linear network forward).

Math: vals = x @ W + b  ([65536,256] @ [256,128] + [128]), then a 7-level
alternating min/max pairwise tree over the 128 leaf columns -> [B, 1].

Strategy (8 NeuronCores, data-parallel over the batch; per-core shard 8192):
  W-stationary matmul orientation. Per core the PE computes
  psum[leaf, batch] = Wh^T @ xh (two K-halves accumulating), so the small
  W[128,128] half is the stationary operand and the 8192 batch columns
  stream through -- the per-matmul LDWEIGHTS cost drops from one 128-row
  load per 128 batch rows (the old x-stationary layout, ~50% PE overhead)
  to one per 512-col PSUM bank (~20%).  Bias becomes a per-PARTITION
  constant in this orientation, so it rides the ACT eviction for free
  (activation Identity with a [128,1] bias AP) and the old bias-seed
  matmuls (~8k PE cycles) disappear.

  Tree: leaves are host-permuted into bit-reversed order, so the deepest
  level pairs leaves (p, p+64) across partitions: DVE computes
  L1 = min(vb[0:64], vb[64:128]) straight on the evicted fp16 data.  The
  halved [64, batch] result is transposed back to [batch, leaf] by PE
  transpose matmuls (64 blocks of [64,128] -> [128,64], identity rhs), and
  the remaining 6 levels run on the free dim at full 128-lane DVE
  utilization, exactly like the old kernel's tree.

  DMA: x is host-prepped fp16 in a chunk-major layout [4 super-chunks x
  2 K-halves x 128 x 2048] so every x load is one fully contiguous 512KB
  read; K-half-0 loads issue on the sync ring and K-half-1 on the gpsimd
  ring so both halves of a chunk land in parallel.

  Output staged as [128, 64] f32 (out[p, c] = batch row 128*c+p),
  de-interleaved on the host.
"""

import numpy as np

try:
    import concourse.bass as bass
except ImportError:  # pragma: no cover
    import sys

    sys.path.insert(0, "/opt/trn_rl_repo")
    import concourse.bass as bass

import concourse.mybir as mybir
import concourse.tile as tile
from concourse import bacc
from concourse.bass_utils import run_bass_kernel_spmd

F32 = mybir.dt.float32
F16 = mybir.dt.float16

B, F, NL = 65536, 256, 128
NCORES = 8
BS = B // NCORES  # 8192 batch rows per core

SUP = 2048  # batch cols per super-chunk (one contiguous 512KB DMA per K-half)
NSUP = BS // SUP  # 4
SUB = 1024  # batch cols per PSUM group (2 banks)
NSUB = SUP // SUB  # 2

# Tree ops, deepest level first (palindrome list: min,max,min,max,min,max,min)
_TREE_OPS = [
    mybir.AluOpType.min if i % 2 == 0 else mybir.AluOpType.max for i in range(7)
]


def _bitrev7_perm() -> np.ndarray:
    perm = np.zeros(NL, dtype=np.int64)
    for p in range(NL):
        r = 0
        for k in range(7):
            r |= ((p >> k) & 1) << (6 - k)
        perm[p] = r
    return perm


def build_nc(bs: int = BS):
    nc = bacc.Bacc(None)
    # chunk-major x: row (s*256 + h*128 + k) col c  <->  xT[h*128+k, s*SUP+c]
    xd = nc.declare_dram_parameter("xd", [NSUP * 256, SUP], F16, isOutput=False)
    Wp = nc.declare_dram_parameter("Wp", [F, NL], F16, isOutput=False)
    bc = nc.declare_dram_parameter("bc", [128, 1], F32, isOutput=False)
    ncols = bs // 128  # 64
    out = nc.declare_dram_parameter("out", [128, ncols], F32, isOutput=True)

    with tile.TileContext(nc, pool_alloc_mode="queue") as tc:
        with (
            tc.tile_pool(name="const", bufs=1) as cpool,
            tc.tile_pool(name="xin", bufs=1) as xpool,
            tc.tile_pool(name="psum", bufs=2, space=bass.MemorySpace.PSUM) as ppool,
            tc.tile_pool(name="sb", bufs=1) as spool,
        ):
            # constants ride the scalar ring so the big x loads (sync +
            # gpsimd rings) start immediately
            w0t = cpool.tile([128, NL], F16, tag="w0t")
            w1t = cpool.tile([128, NL], F16, tag="w1t")
            bct = cpool.tile([128, 1], F32, tag="bct")
            nc.scalar.dma_start(out=w0t[:], in_=Wp[0:128, :])
            nc.scalar.dma_start(out=w1t[:], in_=Wp[128:256, :])
            nc.scalar.dma_start(out=bct[:], in_=bc[:])

            # x super-chunk loads: K-half 0 on the sync HWDGE ring, K-half 1
            # on the gpsimd SWDGE ring, so both halves stream in parallel
            x0s, x1s = [], []
            for s in range(NSUP):
                x0 = xpool.tile([128, SUP], F16, tag=f"x0_{s}")
                x1 = xpool.tile([128, SUP], F16, tag=f"x1_{s}")
                nc.sync.dma_start(out=x0[:], in_=xd[s * 256 : s * 256 + 128, :])
                nc.gpsimd.dma_start(
                    out=x1[:], in_=xd[s * 256 + 128 : s * 256 + 256, :]
                )
                x0s.append(x0)
                x1s.append(x1)

            # flat SBUF intermediates (no buffer recycling -> no false deps)
            vb = spool.tile([128, bs], F16, tag="vb")  # evicted vals [leaf, batch]
            vt = spool.tile([128, bs], F16, tag="vt")  # transposed [batch_p, blk*128]
            l1 = spool.tile([128, bs // 2], F16, tag="l1")
            lvl_tiles = []
            w = 32
            n = bs // 4
            while w >= 2:
                lvl_tiles.append(
                    spool.tile([128, n], F16, tag=f"lv{w}", name=f"lv{w}")
                )
                w //= 2
                n //= 2
            ost = spool.tile([128, ncols], F32, tag="ost")

            for s in range(NSUP):
                ps = ppool.tile([128, SUP], F32, tag="ps", name=f"ps_{s}")
                for bank in range(SUP // 512):
                    c0 = bank * 512
                    nc.tensor.matmul(
                        ps[:, bass.ts(bank, 512)],
                        w0t[:],
                        x0s[s][:, c0 : c0 + 512],
                        start=True,
                        stop=False,
                    )
                for bank in range(SUP // 512):
                    c0 = bank * 512
                    nc.tensor.matmul(
                        ps[:, bass.ts(bank, 512)],
                        w1t[:],
                        x1s[s][:, c0 : c0 + 512],
                        start=False,
                        stop=True,
                    )
                sl = slice(s * SUP, (s + 1) * SUP)
                # fused eviction + bias: vb = Identity(psum * 1 + b[leaf])
                nc.scalar.activation(
                    vb[:, sl],
                    ps[:],
                    mybir.ActivationFunctionType.Identity,
                    bias=bct[:],
                    scale=1.0,
                )
                # DMA XBAR block transpose: vt[p, blk, l] = vb[l, blk*128+p]
                nc.sync.dma_start(
                    out=vt[:, sl].rearrange("p (blk l) -> p blk l", l=128),
                    in_=vb[:, sl],
                    transpose=True,
                )
                # deepest tree level on the free dim
                nc.vector.tensor_tensor(
                    out=l1[:, s * (SUP // 2) : (s + 1) * (SUP // 2)].rearrange(
                        "p (blk h) -> p blk h", h=64
                    ),
                    in0=vt[:, sl].rearrange("p (blk two h) -> p blk two h", two=2, h=64)[
                        :, :, 0, :
                    ],
                    in1=vt[:, sl].rearrange("p (blk two h) -> p blk two h", two=2, h=64)[
                        :, :, 1, :
                    ],
                    op=_TREE_OPS[0],
                )

            # levels 2..7, batched over half-batch groups (32 blocks each)
            for g in range(2):
                cur = l1[:, g * (bs // 4) : (g + 1) * (bs // 4)]
                w = 32
                for lvl in range(1, 7):
                    r = cur.rearrange("p (blk two h) -> p blk two h", two=2, h=w)
                    if lvl < 6:
                        nxt = lvl_tiles[lvl - 1][
                            :, g * (bs // 8 >> (lvl - 1)) : (g + 1) * (bs // 8 >> (lvl - 1))
                        ]
                        outap = nxt.rearrange("p (blk h) -> p blk h", h=w)
                    else:
                        nxt = None
                        outap = ost[:, g * 32 : (g + 1) * 32].rearrange(
                            "p (blk h) -> p blk h", h=1
                        )
                    nc.vector.tensor_tensor(
                        out=outap, in0=r[:, :, 0, :], in1=r[:, :, 1, :], op=_TREE_OPS[lvl]
                    )
                    cur = nxt
                    w //= 2

            nc.sync.dma_start(out=out[:], in_=ost[:])

    nc.compile()
    return nc


_NC_CACHE: dict = {}


def _get_nc(bs=BS):
    if bs not in _NC_CACHE:
        _NC_CACHE[bs] = build_nc(bs)
    return _NC_CACHE[bs]


def prep_inputs(x: np.ndarray, W: np.ndarray, b: np.ndarray) -> list[dict]:
    perm = _bitrev7_perm()
    Wp = np.ascontiguousarray(W[:, perm]).astype(np.float16)
    bc = np.ascontiguousarray(b[perm].astype(np.float32).reshape(128, 1))
    x = np.asarray(x, dtype=np.float32)
    in_maps = []
    for i in range(NCORES):
        xi = x[i * BS : (i + 1) * BS, :].astype(np.float16)
        # [s, col, h, k] -> [s, h, k, col] -> [NSUP*256, SUP]
        xc = np.ascontiguousarray(
            xi.reshape(NSUP, SUP, 2, 128).transpose(0, 2, 3, 1).reshape(NSUP * 256, SUP)
        )
        in_maps.append({"xd": xc, "Wp": Wp, "bc": bc})
    return in_maps


def gather_outputs(results: list[dict]) -> np.ndarray:
    shards = []
    for i in range(NCORES):
        o = np.asarray(results[i]["out"])  # [128, BS//128]; o[p, c] = row 128c+p
        shards.append(o.T.reshape(BS))
    return np.concatenate(shards).reshape(B, 1).astype(np.float32)


def _setup_tracing():
    """Install the antenv.axon_hooks NTFF-profile shim (missing from this
    image) and neuter the artifact upload so traced runs stay local."""
    import sys as _sys
    import types

    import concourse.bass_utils as bu

    bu.upload_artifacts = lambda tmpdir: tmpdir
    try:
        from antenv.axon_hooks import get_axon_ntff_profile_hook  # noqa: F401

        return
    except ImportError:
        pass
    import antenv

    m = types.ModuleType("antenv.axon_hooks")
    _state = {"hook": None}
    m.set_axon_ntff_profile_hook = lambda h: _state.__setitem__("hook", h)
    m.get_axon_ntff_profile_hook = lambda: _state["hook"]
    _sys.modules["antenv.axon_hooks"] = m
    antenv.axon_hooks = m
    try:
        from trn_agent_boot.trn_boot import _ntff_profile_via_ctypes

        hook = _ntff_profile_via_ctypes("/opt/axon/libaxon_pjrt.so")
        if hook is not None:
            m.set_axon_ntff_profile_hook(hook)
    except Exception as e:  # pragma: no cover
        print("ntff hook install failed:", e)


def run_on_hw(x, W, b, trace: bool = False, **kwargs):
    if trace:
        _setup_tracing()
    nc = _get_nc()
    in_maps = prep_inputs(np.asarray(x), np.asarray(W), np.asarray(b))
    return run_bass_kernel_spmd(
        nc, in_maps, core_ids=list(range(NCORES)), trace=trace, **kwargs
    )


def kernel(x: np.ndarray, W: np.ndarray, b: np.ndarray) -> np.ndarray:
    res = run_on_hw(x, W, b, trace=False)
    return gather_outputs(res.results)


# revision 9
# speedup vs baseline: 1.0218x; 1.0218x over previous
"""Trainium2 Bass kernel for nn_ALNet (adaptive linear network forward).

Math: vals = x @ W + b  ([65536,256] @ [256,128] + [128]), then a 7-level
alternating min/max pairwise tree over the 128 leaf columns -> [B, 1].

Strategy (8 NeuronCores, data-parallel over the batch; per-core shard 8192):
  W-stationary matmul orientation. Per core the PE computes
  psum[leaf, batch] = Wh^T @ xh (two K-halves accumulating), so the small
  W[128,128] half is the stationary operand and the 8192 batch columns
  stream through -- the per-matmul LDWEIGHTS cost drops from one 128-row
  load per 128 batch rows (the old x-stationary layout, ~50% PE overhead)
  to one per 512-col PSUM bank (~20%).  Bias becomes a per-PARTITION
  constant in this orientation, so it rides the ACT eviction for free
  (activation Identity with a [128,1] bias AP) and the old bias-seed
  matmuls (~8k PE cycles) disappear.

  Tree: leaves are host-permuted into bit-reversed order, so the deepest
  level pairs leaves (p, p+64) across partitions: DVE computes
  L1 = min(vb[0:64], vb[64:128]) straight on the evicted fp16 data.  The
  halved [64, batch] result is transposed back to [batch, leaf] by PE
  transpose matmuls (64 blocks of [64,128] -> [128,64], identity rhs), and
  the remaining 6 levels run on the free dim at full 128-lane DVE
  utilization, exactly like the old kernel's tree.

  DMA: x is host-prepped fp16 in a chunk-major layout [4 super-chunks x
  2 K-halves x 128 x 2048] so every x load is one fully contiguous 512KB
  read; K-half-0 loads issue on the sync ring and K-half-1 on the gpsimd
  ring so both halves of a chunk land in parallel.

  Output staged as [128, 64] f32 (out[p, c] = batch row 128*c+p),
  de-interleaved on the host.
"""

import numpy as np

try:
    import concourse.bass as bass
except ImportError:  # pragma: no cover
    import sys

    sys.path.insert(0, "/opt/trn_rl_repo")
    import concourse.bass as bass

import concourse.mybir as mybir
import concourse.tile as tile
from concourse import bacc
from concourse.bass_utils import run_bass_kernel_spmd

F32 = mybir.dt.float32
F16 = mybir.dt.float16

B, F, NL = 65536, 256, 128
NCORES = 8
BS = B // NCORES  # 8192 batch rows per core

SUP = 2048  # batch cols per super-chunk (one contiguous 512KB DMA per K-half)
NSUP = BS // SUP  # 4
SUB = 1024  # batch cols per PSUM group (2 banks)
NSUB = SUP // SUB  # 2

# Tree ops, deepest level first (palindrome list: min,max,min,max,min,max,min)
_TREE_OPS = [
    mybir.AluOpType.min if i % 2 == 0 else mybir.AluOpType.max for i in range(7)
]


def _bitrev7_perm() -> np.ndarray:
    perm = np.zeros(NL, dtype=np.int64)
    for p in range(NL):
        r = 0
        for k in range(7):
            r |= ((p >> k) & 1) << (6 - k)
        perm[p] = r
    return perm


def build_nc(bs: int = BS):
    nc = bacc.Bacc(None)
    # chunk-major x: row (s*256 + h*128 + k) col c  <->  xT[h*128+k, s*SUP+c]
    xd = nc.declare_dram_parameter("xd", [NSUP * 256, SUP], F16, isOutput=False)
    Wp = nc.declare_dram_parameter("Wp", [F, NL], F16, isOutput=False)
    bc = nc.declare_dram_parameter("bc", [128, 1], F32, isOutput=False)
    ncols = bs // 128  # 64
    out = nc.declare_dram_parameter("out", [128, ncols], F32, isOutput=True)

    with tile.TileContext(nc, pool_alloc_mode="queue") as tc:
        with (
            tc.tile_pool(name="const", bufs=1) as cpool,
            tc.tile_pool(name="xin", bufs=1) as xpool,
            tc.tile_pool(name="psum", bufs=2, space=bass.MemorySpace.PSUM) as ppool,
            tc.tile_pool(name="sb", bufs=1) as spool,
        ):
            # constants on the sync ring (tiny transfers, land first);
            # K-half-0 sup loads follow on sync, K-half-1 rides the scalar
            # HWDGE ring as two double-sup batched loads so both x halves
            # stream through parallel queues
            w0t = cpool.tile([128, NL], F16, tag="w0t")
            w1t = cpool.tile([128, NL], F16, tag="w1t")
            bct = cpool.tile([128, 1], F32, tag="bct")
            nc.sync.dma_start(out=w0t[:], in_=Wp[0:128, :])
            nc.sync.dma_start(out=w1t[:], in_=Wp[128:256, :])
            nc.sync.dma_start(out=bct[:], in_=bc[:])

            x0s, x1s = [], []
            for s in range(NSUP):
                x0 = xpool.tile([128, SUP], F16, tag=f"x0_{s}")
                nc.sync.dma_start(out=x0[:], in_=xd[s * 256 : s * 256 + 128, :])
                x0s.append(x0)
            for sp in range(NSUP // 2):
                x1 = xpool.tile([128, 2 * SUP], F16, tag=f"x1_{sp}")
                src = xd[:].rearrange("(s h p) c -> p (s h) c", h=2, p=128)
                nc.scalar.dma_start(
                    out=x1[:].rearrange("p (s c) -> p s c", s=2),
                    in_=src[:, 4 * sp + 1 : 4 * sp + 4 : 2, :],
                )
                x1s.append(x1)

            # flat SBUF intermediates (no buffer recycling -> no false deps)
            vb = spool.tile([128, bs], F16, tag="vb")  # evicted vals [leaf, batch]
            vt = spool.tile([128, bs], F16, tag="vt")  # transposed [batch_p, blk*128]
            l1 = spool.tile([128, bs // 2], F16, tag="l1")
            lvl_tiles = []
            w = 32
            n = bs // 4
            while w >= 2:
                lvl_tiles.append(
                    spool.tile([128, n], F16, tag=f"lv{w}", name=f"lv{w}")
                )
                w //= 2
                n //= 2
            ost = spool.tile([128, ncols], F32, tag="ost")

            for s in range(NSUP):
                ps = ppool.tile([128, SUP], F32, tag="ps", name=f"ps_{s}")
                for bank in range(SUP // 512):
                    c0 = bank * 512
                    nc.tensor.matmul(
                        ps[:, bass.ts(bank, 512)],
                        w0t[:],
                        x0s[s][:, c0 : c0 + 512],
                        start=True,
                        stop=False,
                    )
                for bank in range(SUP // 512):
                    c0 = (s % 2) * SUP + bank * 512
                    nc.tensor.matmul(
                        ps[:, bass.ts(bank, 512)],
                        w1t[:],
                        x1s[s // 2][:, c0 : c0 + 512],
                        start=False,
                        stop=True,
                    )
                sl = slice(s * SUP, (s + 1) * SUP)
                # fused eviction + bias (vb = psum + b[leaf]): alternate the
                # ACT and gpsimd engines so neither becomes the bottleneck
                if s % 2 == 0:
                    nc.scalar.activation(
                        vb[:, sl],
                        ps[:],
                        mybir.ActivationFunctionType.Identity,
                        bias=bct[:],
                        scale=1.0,
                    )
                else:
                    nc.vector.tensor_scalar(
                        out=vb[:, sl],
                        in0=ps[:],
                        scalar1=bct[:],
                        scalar2=None,
                        op0=mybir.AluOpType.add,
                    )
                # DMA XBAR block transpose: vt[p, blk, l] = vb[l, blk*128+p],
                # alternating queues so transposes overlap
                treng = nc.scalar if s % 2 == 0 else nc.sync
                treng.dma_start(
                    out=vt[:, sl].rearrange("p (blk l) -> p blk l", l=128),
                    in_=vb[:, sl],
                    transpose=True,
                )
                # deepest tree level on the free dim
                nc.vector.tensor_tensor(
                    out=l1[:, s * (SUP // 2) : (s + 1) * (SUP // 2)].rearrange(
                        "p (blk h) -> p blk h", h=64
                    ),
                    in0=vt[:, sl].rearrange("p (blk two h) -> p blk two h", two=2, h=64)[
                        :, :, 0, :
                    ],
                    in1=vt[:, sl].rearrange("p (blk two h) -> p blk two h", two=2, h=64)[
                        :, :, 1, :
                    ],
                    op=_TREE_OPS[0],
                )

            # levels 2..7, batched over half-batch groups (32 blocks each)
            for g in range(2):
                cur = l1[:, g * (bs // 4) : (g + 1) * (bs // 4)]
                w = 32
                for lvl in range(1, 7):
                    r = cur.rearrange("p (blk two h) -> p blk two h", two=2, h=w)
                    if lvl < 6:
                        nxt = lvl_tiles[lvl - 1][
                            :, g * (bs // 8 >> (lvl - 1)) : (g + 1) * (bs // 8 >> (lvl - 1))
                        ]
                        outap = nxt.rearrange("p (blk h) -> p blk h", h=w)
                    else:
                        nxt = None
                        outap = ost[:, g * 32 : (g + 1) * 32].rearrange(
                            "p (blk h) -> p blk h", h=1
                        )
                    nc.vector.tensor_tensor(
                        out=outap, in0=r[:, :, 0, :], in1=r[:, :, 1, :], op=_TREE_OPS[lvl]
                    )
                    cur = nxt
                    w //= 2

            nc.sync.dma_start(out=out[:], in_=ost[:])

    nc.compile()
    return nc


_NC_CACHE: dict = {}


def _get_nc(bs=BS):
    if bs not in _NC_CACHE:
        _NC_CACHE[bs] = build_nc(bs)
    return _NC_CACHE[bs]


def prep_inputs(x: np.ndarray, W: np.ndarray, b: np.ndarray) -> list[dict]:
    perm = _bitrev7_perm()
    Wp = np.ascontiguousarray(W[:, perm]).astype(np.float16)
    bc = np.ascontiguousarray(b[perm].astype(np.float32).reshape(128, 1))
    x = np.asarray(x, dtype=np.float32)
    in_maps = []
    for i in range(NCORES):
        xi = x[i * BS : (i + 1) * BS, :].astype(np.float16)
        # [s, col, h, k] -> [s, h, k, col] -> [NSUP*256, SUP]
        xc = np.ascontiguousarray(
            xi.reshape(NSUP, SUP, 2, 128).transpose(0, 2, 3, 1).reshape(NSUP * 256, SUP)
        )
        in_maps.append({"xd": xc, "Wp": Wp, "bc": bc})
    return in_maps


def gather_outputs(results: list[dict]) -> np.ndarray:
    shards = []
    for i in range(NCORES):
        o = np.asarray(results[i]["out"])  # [128, BS//128]; o[p, c] = row 128c+p
        shards.append(o.T.reshape(BS))
    return np.concatenate(shards).reshape(B, 1).astype(np.float32)


def _setup_tracing():
    """Install the antenv.axon_hooks NTFF-profile shim (missing from this
    image) and neuter the artifact upload so traced runs stay local."""
    import sys as _sys
    import types

    import concourse.bass_utils as bu

    bu.upload_artifacts = lambda tmpdir: tmpdir
    try:
        from antenv.axon_hooks import get_axon_ntff_profile_hook  # noqa: F401

        return
    except ImportError:
        pass
    import antenv

    m = types.ModuleType("antenv.axon_hooks")
    _state = {"hook": None}
    m.set_axon_ntff_profile_hook = lambda h: _state.__setitem__("hook", h)
    m.get_axon_ntff_profile_hook = lambda: _state["hook"]
    _sys.modules["antenv.axon_hooks"] = m
    antenv.axon_hooks = m
    try:
        from trn_agent_boot.trn_boot import _ntff_profile_via_ctypes

        hook = _ntff_profile_via_ctypes("/opt/axon/libaxon_pjrt.so")
        if hook is not None:
            m.set_axon_ntff_profile_hook(hook)
    except Exception as e:  # pragma: no cover
        print("ntff hook install failed:", e)


def run_on_hw(x, W, b, trace: bool = False, **kwargs):
    if trace:
        _setup_tracing()
    nc = _get_nc()
    in_maps = prep_inputs(np.asarray(x), np.asarray(W), np.asarray(b))
    return run_bass_kernel_spmd(
        nc, in_maps, core_ids=list(range(NCORES)), trace=trace, **kwargs
    )


def kernel(x: np.ndarray, W: np.ndarray, b: np.ndarray) -> np.ndarray:
    res = run_on_hw(x, W, b, trace=False)
    return gather_outputs(res.results)


# revision 14
# speedup vs baseline: 1.0377x; 1.0156x over previous
"""Trainium2 Bass kernel for nn_ALNet (adaptive linear network forward).

Math: vals = x @ W + b  ([65536,256] @ [256,128] + [128]), then a 7-level
alternating min/max pairwise tree over the 128 leaf columns -> [B, 1].

Strategy (8 NeuronCores, data-parallel over the batch; per-core shard 8192):
  W-stationary matmul orientation. Per core the PE computes
  psum[leaf, batch] = Wh^T @ xh (two K-halves accumulating), so the small
  W[128,128] half is the stationary operand and the 8192 batch columns
  stream through -- the per-matmul LDWEIGHTS cost drops from one 128-row
  load per 128 batch rows (the old x-stationary layout, ~50% PE overhead)
  to one per 512-col PSUM bank (~20%).  Bias becomes a per-PARTITION
  constant in this orientation, so it rides the ACT eviction for free
  (activation Identity with a [128,1] bias AP) and the old bias-seed
  matmuls (~8k PE cycles) disappear.

  Tree: leaves are host-permuted into bit-reversed order, so the deepest
  level pairs leaves (p, p+64) across partitions: DVE computes
  L1 = min(vb[0:64], vb[64:128]) straight on the evicted fp16 data.  The
  halved [64, batch] result is transposed back to [batch, leaf] by PE
  transpose matmuls (64 blocks of [64,128] -> [128,64], identity rhs), and
  the remaining 6 levels run on the free dim at full 128-lane DVE
  utilization, exactly like the old kernel's tree.

  DMA: x is host-prepped fp16 in a chunk-major layout [4 super-chunks x
  2 K-halves x 128 x 2048] so every x load is one fully contiguous 512KB
  read; K-half-0 loads issue on the sync ring and K-half-1 on the gpsimd
  ring so both halves of a chunk land in parallel.

  Output staged as [128, 64] f32 (out[p, c] = batch row 128*c+p),
  de-interleaved on the host.
"""

import numpy as np

try:
    import concourse.bass as bass
except ImportError:  # pragma: no cover
    import sys

    sys.path.insert(0, "/opt/trn_rl_repo")
    import concourse.bass as bass

import concourse.mybir as mybir
import concourse.tile as tile
from concourse import bacc
from concourse.bass_utils import run_bass_kernel_spmd

F32 = mybir.dt.float32
F16 = mybir.dt.float16

B, F, NL = 65536, 256, 128
NCORES = 8
BS = B // NCORES  # 8192 batch rows per core

SUP = 2048  # batch cols per super-chunk (one contiguous 512KB DMA per K-half)
NSUP = BS // SUP  # 4
SUB = 1024  # batch cols per PSUM group (2 banks)
NSUB = SUP // SUB  # 2

# Tree ops, deepest level first (palindrome list: min,max,min,max,min,max,min)
_TREE_OPS = [
    mybir.AluOpType.min if i % 2 == 0 else mybir.AluOpType.max for i in range(7)
]


def _bitrev7_perm() -> np.ndarray:
    perm = np.zeros(NL, dtype=np.int64)
    for p in range(NL):
        r = 0
        for k in range(7):
            r |= ((p >> k) & 1) << (6 - k)
        perm[p] = r
    return perm


def build_nc(bs: int = BS):
    nc = bacc.Bacc(None)
    # chunk-major x: row (s*256 + h*128 + k) col c  <->  xT[h*128+k, s*SUP+c]
    xd = nc.declare_dram_parameter("xd", [NSUP * 256, SUP], F16, isOutput=False)
    # packed W: cols 0:128 = K-half-0, 128:256 = K-half-1
    cst = nc.declare_dram_parameter("cst", [128, 256], F16, isOutput=False)
    bc = nc.declare_dram_parameter("bc", [128, 1], F32, isOutput=False)
    ncols = bs // 128  # 64
    out = nc.declare_dram_parameter("out", [128, ncols], F32, isOutput=True)

    CH = 1024  # psum chunk (2 banks)
    NCH = bs // CH  # 8
    # eviction engine per chunk: DVE early chunks + chunk 6 (parallel with
    # ACT's chunk 7 so the tail transposes start back-to-back), ACT the rest
    DVE_EVICT = {0, 1, 6}

    with tile.TileContext(nc, pool_alloc_mode="queue") as tc:
        with (
            tc.tile_pool(name="const", bufs=1) as cpool,
            tc.tile_pool(name="xin", bufs=1) as xpool,
            tc.tile_pool(name="psum", bufs=3, space=bass.MemorySpace.PSUM) as ppool,
            tc.tile_pool(name="warm", bufs=1, space=bass.MemorySpace.PSUM) as wpool,
            tc.tile_pool(name="sb", bufs=1) as spool,
        ):
            cstt = cpool.tile([128, 256], F16, tag="cstt")
            bctt = cpool.tile([128, 1], F32, tag="bctt")
            nc.scalar.dma_start(out=cstt[:], in_=cst[:])
            nc.scalar.dma_start(out=bctt[:], in_=bc[:])
            w0t = cstt[:, 0:128]
            w1t = cstt[:, 128:256]
            bct = bctt[:]

            # x loads: K-half 0 on sync (small head pieces so the PE starts
            # ASAP), K-half 1 on scalar
            xr = xd[:].rearrange("(s h p) c -> p s h c", h=2, p=128)
            # (start_col, len, src AP) pieces; boundaries align to sups
            x0_parts = [
                (0, 1024, xr[:, 0, 0, 0:1024]),
                (1024, 1024, xr[:, 0, 0, 1024:2048]),
                (2048, 2048, xr[:, 1, 0, :]),
                (4096, 2048, xr[:, 2, 0, :]),
                (6144, 2048, xr[:, 3, 0, :]),
            ]
            x1_parts = [
                (0, 2048, xr[:, 0, 1, :]),
                (2048, 2048, xr[:, 1, 1, :]),
                (4096, 4096, xr[:, 2:4, 1, :]),
            ]
            x0s, x1s = [], []
            for i, (r0, ln, src0) in enumerate(x0_parts):
                t = xpool.tile([128, ln], F16, tag=f"x0_{i}")
                nc.sync.dma_start(out=t[:], in_=src0)
                x0s.append((t, r0))
            for i, (r1, ln, src1) in enumerate(x1_parts):
                t = xpool.tile([128, ln], F16, tag=f"x1_{i}")
                nc.scalar.dma_start(out=t[:], in_=src1)
                x1s.append((t, r1))

            def xslice(parts, c0, ln):
                for t, base in parts:
                    if base <= c0 and c0 + ln <= base + t.shape[1]:
                        return t[:, c0 - base : c0 - base + ln]
                raise AssertionError("bad x slice")

            # flat SBUF intermediates
            vb = spool.tile([128, bs], F16, tag="vb")  # [leaf, batch]
            vt = spool.tile([128, bs], F16, tag="vt")  # [batch_p, blk*128]
            l1 = spool.tile([128, bs // 2], F16, tag="l1")
            lvl_tiles = []
            w = 32
            n = bs // 4
            while w >= 2:
                lvl_tiles.append(
                    spool.tile([128, n], F16, tag=f"lv{w}", name=f"lv{w}")
                )
                w //= 2
                n //= 2
            ost = spool.tile([128, ncols], F32, tag="ost")

            # PE p-state warmup: garbage matmuls with no data deps keep the
            # PE streaming from the preamble until real x data lands, so the
            # real matmuls run at full clock
            garb = spool.tile([128, 512], F16, tag="garb")
            warm = wpool.tile([128, 512], F32, tag="warm")
            nc.gpsimd.memset(garb[:], 0.0)
            for i in range(10):
                nc.tensor.matmul(
                    warm[:], garb[:, 0:128], garb[:], start=True, stop=True,
                )

            def evict(c):
                ps = pss[c]
                slc = slice(c * CH, (c + 1) * CH)
                if c in DVE_EVICT:
                    nc.vector.tensor_scalar(
                        out=vb[:, slc], in0=ps[:], scalar1=bct,
                        scalar2=None, op0=mybir.AluOpType.add,
                    )
                else:
                    nc.scalar.activation(
                        vb[:, slc], ps[:],
                        mybir.ActivationFunctionType.Identity,
                        bias=bct, scale=1.0,
                    )

            def tr(eng, c0, ln):
                eng.dma_start(
                    out=vt[:, c0 : c0 + ln].rearrange(
                        "p (blk l) -> p blk l", l=128
                    ),
                    in_=vb[:, c0 : c0 + ln],
                    transpose=True,
                )

            def l1op(c0, ln):
                rr = vt[:, c0 : c0 + ln].rearrange(
                    "p (blk two h) -> p blk two h", two=2, h=64
                )
                nc.vector.tensor_tensor(
                    out=l1[:, c0 // 2 : (c0 + ln) // 2].rearrange(
                        "p (blk h) -> p blk h", h=64
                    ),
                    in0=rr[:, :, 0, :], in1=rr[:, :, 1, :], op=_TREE_OPS[0],
                )

            def tree(g0, gn):
                # levels 2..7 for batch blocks [g0*8, (g0+gn)*8)
                cur = l1[:, g0 * (CH // 2) : (g0 + gn) * (CH // 2)]
                w = 32
                nblk = gn * 8
                for lvl in range(1, 7):
                    r = cur.rearrange("p (blk two h) -> p blk two h", two=2, h=w)
                    if lvl < 6:
                        base = lvl_tiles[lvl - 1]
                        seg = (CH // 4) >> (lvl - 1)
                        nxt = base[:, g0 * seg : (g0 + gn) * seg]
                        outap = nxt.rearrange("p (blk h) -> p blk h", h=w)
                    else:
                        nxt = None
                        outap = ost[:, g0 * 8 : (g0 + gn) * 8].rearrange(
                            "p (blk h) -> p blk h", h=1
                        )
                    nc.vector.tensor_tensor(
                        out=outap, in0=r[:, :, 0, :], in1=r[:, :, 1, :],
                        op=_TREE_OPS[lvl],
                    )
                    cur = nxt
                    w //= 2

            pss = {}
            for c in range(NCH):
                ps = ppool.tile([128, CH], F32, tag="ps", name=f"ps_{c}")
                pss[c] = ps
                for bank in range(CH // 512):
                    c0 = c * CH + bank * 512
                    nc.tensor.matmul(
                        ps[:, bass.ts(bank, 512)], w0t,
                        xslice(x0s, c0, 512), start=True, stop=False,
                    )
                for bank in range(CH // 512):
                    c0 = c * CH + bank * 512
                    nc.tensor.matmul(
                        ps[:, bass.ts(bank, 512)], w1t,
                        xslice(x1s, c0, 512), start=False, stop=True,
                    )
                evict(c)
                # transposes: full sups for 0..2 (alternating queues), the
                # last sup in halves so the tail chain is short
                if c == 1:
                    tr(nc.scalar, 0, 2048)
                    l1op(0, 2048)
                elif c == 3:
                    tr(nc.sync, 2048, 2048)
                    l1op(2048, 2048)
                    tree(0, 4)
                elif c == 5:
                    tr(nc.scalar, 4096, 2048)
                    l1op(4096, 2048)
                elif c == 6:
                    tr(nc.sync, 6144, 1024)
                    l1op(6144, 1024)
                    tree(4, 2)
                elif c == 7:
                    tr(nc.sync, 7168, 1024)
                    l1op(7168, 1024)
                    tree(6, 2)

            nc.sync.dma_start(out=out[:], in_=ost[:])

    nc.compile()
    return nc


_NC_CACHE: dict = {}


def _get_nc(bs=BS):
    if bs not in _NC_CACHE:
        _NC_CACHE[bs] = build_nc(bs)
    return _NC_CACHE[bs]


def prep_inputs(x: np.ndarray, W: np.ndarray, b: np.ndarray) -> list[dict]:
    perm = _bitrev7_perm()
    Wp = np.ascontiguousarray(W[:, perm]).astype(np.float16)
    bc = b[perm].astype(np.float32).reshape(128, 1)
    cst = np.empty((128, 256), dtype=np.float16)
    cst[:, 0:128] = Wp[0:128, :]
    cst[:, 128:256] = Wp[128:256, :]
    cst = np.ascontiguousarray(cst)
    x = np.asarray(x, dtype=np.float32)
    in_maps = []
    for i in range(NCORES):
        xi = x[i * BS : (i + 1) * BS, :].astype(np.float16)
        # [s, col, h, k] -> [s, h, k, col] -> [NSUP*256, SUP]
        xc = np.ascontiguousarray(
            xi.reshape(NSUP, SUP, 2, 128).transpose(0, 2, 3, 1).reshape(NSUP * 256, SUP)
        )
        in_maps.append({"xd": xc, "cst": cst, "bc": bc})
    return in_maps


def gather_outputs(results: list[dict]) -> np.ndarray:
    shards = []
    for i in range(NCORES):
        o = np.asarray(results[i]["out"])  # [128, BS//128]; o[p, c] = row 128c+p
        shards.append(o.T.reshape(BS))
    return np.concatenate(shards).reshape(B, 1).astype(np.float32)


def _setup_tracing():
    """Install the antenv.axon_hooks NTFF-profile shim (missing from this
    image) and neuter the artifact upload so traced runs stay local."""
    import sys as _sys
    import types

    import concourse.bass_utils as bu

    bu.upload_artifacts = lambda tmpdir: tmpdir
    try:
        from antenv.axon_hooks import get_axon_ntff_profile_hook  # noqa: F401

        return
    except ImportError:
        pass
    import antenv

    m = types.ModuleType("antenv.axon_hooks")
    _state = {"hook": None}
    m.set_axon_ntff_profile_hook = lambda h: _state.__setitem__("hook", h)
    m.get_axon_ntff_profile_hook = lambda: _state["hook"]
    _sys.modules["antenv.axon_hooks"] = m
    antenv.axon_hooks = m
    try:
        from trn_agent_boot.trn_boot import _ntff_profile_via_ctypes

        hook = _ntff_profile_via_ctypes("/opt/axon/libaxon_pjrt.so")
        if hook is not None:
            m.set_axon_ntff_profile_hook(hook)
    except Exception as e:  # pragma: no cover
        print("ntff hook install failed:", e)


def run_on_hw(x, W, b, trace: bool = False, **kwargs):
    if trace:
        _setup_tracing()
    nc = _get_nc()
    in_maps = prep_inputs(np.asarray(x), np.asarray(W), np.asarray(b))
    return run_bass_kernel_spmd(
        nc, in_maps, core_ids=list(range(NCORES)), trace=trace, **kwargs
    )


def kernel(x: np.ndarray, W: np.ndarray, b: np.ndarray) -> np.ndarray:
    res = run_on_hw(x, W, b, trace=False)
    return gather_outputs(res.results)
